# revision 1
# baseline (speedup 1.0000x reference)
"""Trainium2 Bass kernel for a dense transformer block (nn_Block_31387620999284).

Sharding: 8 cores = 4 batches x 2 parity groups. Core c handles batch b=c//2
and the query tokens with sequence parity d=c%2. Every core computes K/V for
its batch's full 2048-token sequence, so there is no cross-core communication.
To keep the instruction stream identical across cores (SPMD), odd-parity cores
receive the token sequence with each even/odd pair swapped so that "query
tokens" are always the even positions; the causal-diagonal mask (per-core
input data) absorbs the permutation.

v2 highlights over the original baseline:
- All five big GEMM families (Q/K/V/out-proj/FFN) run in fp8e4m3 with
  perf_mode=DoubleRow (K=256 per matmul), with power-of-2 weight scaling
  folded into the epilogues. Attention (scores, AV) stays bf16.
- LayerNorm1 is computed in token-major layout on the Vector engine (one
  reduce + one tensor_tensor_reduce per 128-token tile) and moved to the
  feature-major layout with SBUF->SBUF DMA transposes, eliminating all
  LN-stats matmuls from the PE.
- Softmax denominators use reciprocal_approx_fast on the PSUM row plus a
  GpSimd partition_broadcast instead of a K=1 PE matmul + full-width DVE
  reciprocal.
- LN2 stats use a DVE pairwise-add tree plus two small bf16 matmuls.
"""

import sys

for _p in ("/opt/trn_rl_repo",):
    if _p not in sys.path:
        sys.path.append(_p)

import numpy as np
import ml_dtypes
from contextlib import ExitStack

import concourse.bass as bass
import concourse.tile as tile
from concourse import bacc, mybir
from concourse.bass import ts
from concourse.bass_utils import run_bass_kernel_spmd


def _install_ntff_hook():
    """The container's antenv stub lacks axon_hooks; provide it so tracing
    (BASS_TRACE=1) works instead of crashing on import."""
    try:
        import antenv.axon_hooks  # noqa: F401
        return
    except ImportError:
        pass
    try:
        import types
        import antenv
        mod = types.ModuleType("antenv.axon_hooks")
        mod._hook = None
        mod.set_axon_ntff_profile_hook = lambda h: setattr(mod, "_hook", h)
        mod.get_axon_ntff_profile_hook = lambda: mod._hook
        sys.modules["antenv.axon_hooks"] = mod
        antenv.axon_hooks = mod
        try:
            from trn_agent_boot.trn_boot import _ntff_profile_via_ctypes
            mod._hook = _ntff_profile_via_ctypes("/opt/axon/libaxon_pjrt.so")
        except Exception:
            pass
    except Exception:
        pass


_install_ntff_hook()

P = 128
D = 1024
TKV = 2048
TQ = 1024
F = 4096
H = 16
HD = 64
DP = D // P    # 8
FP = F // P    # 32
CH = 512       # token chunk / matmul free dim
QB = 512       # attention query block
NQB = TQ // QB # 2
NKT = TKV // P # 16 key tiles
NT = TKV // P  # 16 LN tiles of 128 tokens
EPS = 1e-5

F32 = mybir.dt.float32
BF16 = mybir.dt.bfloat16
FP8 = mybir.dt.float8e4
AF = mybir.ActivationFunctionType
ALU = mybir.AluOpType
DR = mybir.MatmulPerfMode.DoubleRow

# power-of-2 fp8 weight scales (host multiplies weights by 2**K_*, the
# epilogues divide the PSUM result back down)
K_QKV = 12   # |w| <= 1/32 -> max 128
K_O = 12
K_1 = 12
K_2 = 13     # |w2| <= 1/64 -> max 128


import os
KPHASES = int(os.environ.get("KPHASES", "3"))


class _PhaseDone(Exception):
    pass


def build_nc():
    nc = bacc.Bacc("TRN2", target_bir_lowering=False, debug=False)

    xn = nc.dram_tensor("xn", [TKV, D], BF16, kind="ExternalInput").ap()
    xoT = nc.dram_tensor("xoT", [D, TQ], F32, kind="ExternalInput").ap()
    wq = nc.dram_tensor("wq", [D, D], FP8, kind="ExternalInput").ap()
    wk = nc.dram_tensor("wk", [D, D], FP8, kind="ExternalInput").ap()
    wv = nc.dram_tensor("wv", [D, D], FP8, kind="ExternalInput").ap()
    wo = nc.dram_tensor("wo", [D, D], FP8, kind="ExternalInput").ap()
    w1 = nc.dram_tensor("w1", [D, F], FP8, kind="ExternalInput").ap()
    w2 = nc.dram_tensor("w2", [F, D], BF16, kind="ExternalInput").ap()
    # bias columns: b2 8:16 | bq 16:24 | bk 24:32 | b1' 32:64
    biases = nc.dram_tensor("biases", [P, 64], F32, kind="ExternalInput").ap()
    bvr = nc.dram_tensor("bvr", [P, D], F32, kind="ExternalInput").ap()
    mk = nc.dram_tensor("mk", [P, 2, 64], BF16, kind="ExternalInput").ap()
    outT = nc.dram_tensor("outT", [D, TQ], F32, kind="ExternalOutput").ap()

    xn3 = xn.rearrange("(i p) d -> i p d", p=P)        # 16 tiles of 128 tokens
    xoT3 = xoT.rearrange("(o p) t -> p o t", p=P)
    out3 = outT.rearrange("(o p) t -> p o t", p=P)
    wq3 = wq.rearrange("(o p) m -> p o m", p=P)
    wk3 = wk.rearrange("(o p) m -> p o m", p=P)
    wv3 = wv.rearrange("(o p) m -> p o m", p=P)
    wo3 = wo.rearrange("(o p) m -> p o m", p=P)
    w13 = w1.rearrange("(o p) m -> p o m", p=P)
    w23 = w2.rearrange("(o p) m -> p o m", p=P)

    with tile.TileContext(nc) as tc, ExitStack() as ctx:
        consts = ctx.enter_context(tc.tile_pool(name="consts", bufs=1))
        bias_sb = consts.tile([P, 64], F32, name="bias_sb")
        nc.sync.dma_start(bias_sb[:], biases)
        ones_b16 = consts.tile([P, 1], BF16, name="ones_b16")
        nc.vector.memset(ones_b16[:], 1.0)
        ones_f32 = consts.tile([1, P], F32, name="ones_f32")
        nc.vector.memset(ones_f32[:], 1.0)
        # warm the ACT function tables before the LN pipeline needs them (a
        # mid-phase ACT_TABLE_LOAD stalls the strict-FIFO scalar engine)
        wrm = consts.tile([1, 1], F32, name="wrm")
        nc.vector.memset(wrm[:], 1.0)
        wrm2 = consts.tile([1, 1], F32, name="wrm2")
        nc.scalar.activation(wrm2[:], wrm[:], AF.Sqrt)
        nc.scalar.activation(wrm2[:], wrm[:], AF.Exp)
        nc.scalar.activation(wrm2[:], wrm[:], AF.Square)

        # Long-lived right-side pools.
        sWX = ExitStack()
        mskp = sWX.enter_context(tc.tile_pool(name="mskp", bufs=1, side="right"))
        mask_sb = mskp.tile([P, 2, 64], BF16, name="mask_sb")
        nc.sync.dma_start(mask_sb[:], mk)
        wop = sWX.enter_context(tc.tile_pool(name="wop", bufs=1, side="right"))
        wo_sb = wop.tile([P, DP, D], FP8, name="wo_sb")
        xop = sWX.enter_context(tc.tile_pool(name="xop", bufs=3, side="right"))

        # Persistent K/V/Q for attention (phases 1-2).
        sKVQ = ExitStack()
        kvqp = sKVQ.enter_context(tc.tile_pool(name="kvqp", bufs=1))
        KT_all = kvqp.tile([P, DP, TKV], BF16, name="KT_all")
        V_all = kvqp.tile([P, NKT, H, HD + 1], FP8, name="V_all")
        QT_all = kvqp.tile([P, DP, TQ], BF16, name="QT_all")

        # ================= Phase 1: LN1 + Q/K/V projections =================
        with ExitStack() as p1:
            xtp = p1.enter_context(tc.tile_pool(name="xtp", bufs=6))
            lnsp = p1.enter_context(tc.tile_pool(name="lnsp", bufs=6))
            hnp = p1.enter_context(tc.tile_pool(name="hnp", bufs=5))
            hctp = p1.enter_context(tc.tile_pool(name="hctp", bufs=3))
            hc8p = p1.enter_context(tc.tile_pool(name="hc8p", bufs=3))
            q8p = p1.enter_context(tc.tile_pool(name="q8p", bufs=2))
            mmp = p1.enter_context(tc.tile_pool(name="mmp1", bufs=4, space="PSUM"))
            bvp = p1.enter_context(tc.tile_pool(name="bvp", bufs=1))
            wkvp = p1.enter_context(tc.tile_pool(name="wkvp", bufs=1))
            wqp = p1.enter_context(tc.tile_pool(name="wqp", bufs=1, side="right"))

            # bulk weight loads on the gpsimd DMA queue so they don't delay
            # the per-tile x streams on the sync queue
            nc.vector.memset(V_all[:, :, :, HD:HD + 1], 1.0)
            bvr_sb = bvp.tile([P, D], F32, name="bvr_sb")
            nc.gpsimd.dma_start(bvr_sb[:], bvr)
            wk_sb = wkvp.tile([P, DP, D], FP8, name="wk_sb")
            nc.gpsimd.dma_start(wk_sb[:], wk3)
            wv_sb = wkvp.tile([P, DP, D], FP8, name="wv_sb")
            nc.gpsimd.dma_start(wv_sb[:], wv3)
            wq_sb = wqp.tile([P, DP, D], FP8, name="wq_sb")
            nc.gpsimd.dma_start(wq_sb[:], wq3)
            nc.gpsimd.dma_start(wo_sb[:], wo3)

            def ln_tile(i, hcT):
                """LN over one 128-token tile in token-major layout, then
                DMA-transpose into hcT[:, :, (i%4)*128 : ...]."""
                xt = xtp.tile([P, D], BF16, name="xt", tag="xt")
                nc.scalar.dma_start(xt[:], xn3[i])
                s1 = lnsp.tile([P, 1], F32, name="s1", tag="s1")
                nc.vector.tensor_reduce(s1[:], xt[:], axis=mybir.AxisListType.X,
                                        op=ALU.add)
                sqs = xtp.tile([P, D], BF16, name="sqs", tag="sqs")
                s2 = lnsp.tile([P, 1], F32, name="s2", tag="s2")
                # (tensor_tensor_reduce crashes TRN2 hw; ACT Square+accum works)
                nc.scalar.activation(sqs[:], xt[:], AF.Square, accum_out=s2[:])
                negmu = lnsp.tile([P, 1], F32, name="negmu", tag="negmu")
                nc.vector.tensor_scalar_mul(negmu[:], s1[:], -1.0 / D)
                muu = lnsp.tile([P, 1], F32, name="muu", tag="muu")
                nc.vector.tensor_mul(muu[:], negmu[:], negmu[:])
                sdb = lnsp.tile([P, 1], F32, name="sdb", tag="sdb")
                nc.vector.tensor_scalar(sdb[:], muu[:], -1.0, EPS,
                                        op0=ALU.mult, op1=ALU.add)
                sd = lnsp.tile([P, 1], F32, name="sd", tag="sd")
                nc.scalar.activation(sd[:], s2[:], AF.Sqrt, bias=sdb[:],
                                     scale=1.0 / D)
                rstd = lnsp.tile([P, 1], F32, name="rstd", tag="rstd")
                nc.vector.reciprocal(rstd[:], sd[:])
                hN = hnp.tile([P, D], BF16, name="hN", tag="hN")
                nc.gpsimd.tensor_scalar(hN[:], xt[:], negmu[:], rstd[:],
                                        op0=ALU.add, op1=ALU.mult)
                j = i % 4
                nc.sync.dma_start_transpose(hcT[:, :, ts(j, P)], hN[:])

            # chunk list: 4 KV chunks then 2 Q chunks
            chunks = [("kv", c) for c in range(TKV // CH)] + \
                     [("q", c) for c in range(TQ // CH)]
            hc8s = {}
            q8s = {}

            def make_chunk(c):
                """LN tiles + per-tile casts for KV chunk c; produces hc8 (and
                half of a q8 chunk). Casts run on the idle GpSimd engine at
                128-token granularity so consumers can start early."""
                hcT = hctp.tile([P, DP, CH], BF16, name="hcT", tag="hcT")
                hc8 = hc8p.tile([P, DP, CH], FP8, name="hc8", tag="hc8")
                hc8s[c] = hc8
                qc, qh = divmod(c, 2)
                if qh == 0:
                    q8s[qc] = q8p.tile([P, DP, CH], FP8, name="q8", tag="q8")
                q8 = q8s[qc]
                for j in range(4):
                    ln_tile(c * 4 + j, hcT)
                    nc.scalar.activation(hc8[:, :, ts(j, P)], hcT[:, :, ts(j, P)],
                                         AF.Copy)
                    # even token positions of this tile -> q tokens
                    nc.scalar.activation(
                        q8[:, :, qh * (CH // 2) + j * 64:qh * (CH // 2) + (j + 1) * 64],
                        hcT[:, :, j * P:(j + 1) * P:2], AF.Copy)

            make_chunk(0)

            if KPHASES < 1:
                # debug: LN pipeline only; dump hc8 chunks 0..1 as f32
                for c in range(1, 4):
                    make_chunk(c)
                with tc.tile_pool(name="dbg", bufs=2) as dbg:
                    for c in range(2):
                        for i in range(DP):
                            dt_ = dbg.tile([P, CH], F32, name="dt", tag="dt")
                            nc.vector.tensor_copy(dt_[:], hc8s[c][:, i, :])
                            nc.sync.dma_start(out3[:, i, ts(c, CH)], dt_[:])
                chunks = []

            for idx, (kind, c) in enumerate(chunks):
                # run the next chunk's LN work interleaved with this chunk's
                # matmuls (issue order; Tile overlaps them)
                if kind == "kv":
                    hc8 = hc8s.pop(c)
                    if c + 1 < 4:
                        make_chunk(c + 1)

                    def kproj(c=c, hc8=hc8):
                        for hp in range(DP):
                            ps = mmp.tile([P, CH], F32, name="psk", tag="mm1")
                            for ks in range(4):
                                nc.tensor.matmul(ps[:], wk_sb[:, 2 * ks:2 * ks + 2, ts(hp, P)],
                                                 hc8[:, 2 * ks:2 * ks + 2, :],
                                                 start=(ks == 0), stop=(ks == 3),
                                                 perf_mode=DR)
                            nc.vector.tensor_scalar(KT_all[:, hp, ts(c, CH)], ps[:],
                                                    2.0 ** -K_QKV,
                                                    bias_sb[:, 24 + hp:25 + hp],
                                                    op0=ALU.mult, op1=ALU.add)

                    def vproj(c=c, hc8=hc8):
                        # st-outer: each 128-token tile's V finishes before the
                        # next tile's cast is needed (lets chunk 0 start early)
                        for st in range(4):
                            for dc in range(2):
                                ps = mmp.tile([P, CH], F32, name="psv", tag="mm1")
                                for ks in range(4):
                                    nc.tensor.matmul(ps[:], hc8[:, 2 * ks:2 * ks + 2, ts(st, P)],
                                                     wv_sb[:, 2 * ks:2 * ks + 2, ts(dc, CH)],
                                                     start=(ks == 0), stop=(ks == 3),
                                                     perf_mode=DR)
                                vdst = V_all[:, c * 4 + st, dc * 8:dc * 8 + 8, 0:HD]
                                nc.vector.scalar_tensor_tensor(
                                    vdst,
                                    ps[:].rearrange("p (h d) -> p h d", h=8),
                                    2.0 ** -K_QKV,
                                    bvr_sb[:, ts(dc, CH)].rearrange("p (h d) -> p h d", h=8),
                                    op0=ALU.mult, op1=ALU.add)

                    if c == 0:
                        vproj()
                        kproj()
                    else:
                        kproj()
                        vproj()
                else:
                    q8 = q8s.pop(c)
                    for hp in range(DP):
                        ps = mmp.tile([P, CH], F32, name="psq", tag="mm1")
                        for ks in range(4):
                            nc.tensor.matmul(ps[:], wq_sb[:, 2 * ks:2 * ks + 2, ts(hp, P)],
                                             q8[:, 2 * ks:2 * ks + 2, :],
                                             start=(ks == 0), stop=(ks == 3),
                                             perf_mode=DR)
                        nc.vector.tensor_scalar(QT_all[:, hp, ts(c, CH)], ps[:],
                                                2.0 ** -K_QKV,
                                                bias_sb[:, 16 + hp:17 + hp],
                                                op0=ALU.mult, op1=ALU.add)

        if KPHASES < 2:
            # debug: dump KT_all (or QT_all with KDUMP=QT) and stop
            dsrc = QT_all if os.environ.get("KDUMP") == "QT" else KT_all
            with tc.tile_pool(name="dbg", bufs=2) as dbg:
                for i in range(DP):
                    dt_ = dbg.tile([P, TQ], F32, name="dt", tag="dt")
                    nc.vector.tensor_copy(dt_[:], dsrc[:, i, 0:TQ])
                    nc.sync.dma_start(out3[:, i, :], dt_[:])
            sKVQ.close()
            sWX.close()

        if KPHASES >= 2:
            # ============ Phase 2+3: attention with interleaved out-proj ============
            sX2 = ExitStack()
            x2p = sX2.enter_context(tc.tile_pool(name="x2p", bufs=1, side="right"))
            x2T = x2p.tile([P, DP, TQ], F32, name="x2T")
            h28 = x2p.tile([P, DP, TQ], FP8, name="h28")
            sATT = ExitStack()
            attp = sATT.enter_context(tc.tile_pool(name="attp", bufs=1, side="right"))
            attn8 = attp.tile([P, DP, TQ], FP8, name="attn8")
            p3t = ExitStack()       # tp3 outlives p2 pools (used by wo epilogues)
            tp3 = p3t.enter_context(tc.tile_pool(name="tp3", bufs=4, side="right"))

            def wo_group(qc, i, pspool, pstag):
                """One out-projection output tile + residual epilogue -> x2T."""
                ps = pspool.tile([P, CH], F32, name="pso", tag=pstag)
                for ks in range(4):
                    nc.tensor.matmul(ps[:], wo_sb[:, 2 * ks:2 * ks + 2, ts(i, P)],
                                     attn8[:, 2 * ks:2 * ks + 2, ts(qc, CH)],
                                     start=(ks == 0), stop=(ks == 3),
                                     perf_mode=DR)
                xo = xop.tile([P, CH], F32, name="xo", tag="xo")
                nc.sync.dma_start(xo[:], xoT3[:, i, ts(qc, CH)])
                nc.vector.scalar_tensor_tensor(x2T[:, i, ts(qc, CH)], ps[:],
                                               2.0 ** -K_O, xo[:],
                                               op0=ALU.mult,
                                               op1=ALU.add)

            with ExitStack() as p2:
                psS = p2.enter_context(tc.tile_pool(name="psS", bufs=2, space="PSUM"))
                psAV = p2.enter_context(tc.tile_pool(name="psAV", bufs=4, space="PSUM"))
                weip = p2.enter_context(tc.tile_pool(name="weip", bufs=6))
                smal = p2.enter_context(tc.tile_pool(name="smal", bufs=4))

                scale = float(HD) ** -0.5
                pend = []  # deferred softmax-denominator sections

                def flush_den():
                    for t_, hp_, pavs_ in pend:
                        for l in range(2):
                            pb = 64 * l
                            pav = pavs_[l]
                            den = smal.tile([1, QB], F32, name="den", tag="den")
                            nc.vector.tensor_copy(den[:], pav[64:65, :])
                            rec = smal.tile([1, QB], F32, name="rec", tag="rec")
                            nc.vector.reciprocal_approx_fast(rec[:], den[:])
                            rec64 = smal.tile([64, QB], F32, name="rec64", tag="rec64")
                            nc.gpsimd.partition_broadcast(rec64[:], rec[:], channels=64)
                            nc.vector.tensor_mul(attn8[pb:pb + 64, hp_, ts(t_, QB)],
                                                 pav[0:64, :], rec64[:])
                    pend.clear()

                # global software pipeline over (t, hp) units. Scores/exp run
                # per key tile (128 keys); AV consumes PAIRS of key tiles with
                # fp8 DoubleRow matmuls (K=256 per matmul). Softmax weights in
                # fp8 are numerically free: numerator and denominator use the
                # same quantized weights.
                units = [(t, hp) for t in range(NQB) for hp in range(DP)]

                def score_items():
                    for t, hp in units:
                        for kt in range(8 * (t + 1)):
                            yield (t, hp, kt)
                wei8s = {}

                def scores2(t, hp, kt):
                    o = 64 * (kt - 8 * t) if kt >= 8 * t else 0
                    op = 64 * ((kt & ~1) - 8 * t) if (kt & ~1) >= 8 * t else 0
                    ps2 = psS.tile([P, 2, QB], F32, name="pss2", tag="pss2")
                    for l in range(2):
                        pb = 64 * l
                        nc.tensor.matmul(ps2[:, l, o:],
                                         KT_all[pb:pb + 64, hp, ts(kt, P)],
                                         QT_all[pb:pb + 64, hp,
                                                t * QB + o:(t + 1) * QB],
                                         start=True, stop=True)
                    if kt & 1 == 0:
                        wei8s[(t, hp, kt // 2)] = weip.tile(
                            [P, 2, 2, QB], FP8, name="wei8", tag="wei8")
                    wei8 = wei8s[(t, hp, kt // 2)]
                    nc.scalar.activation(wei8[:, kt & 1, :, o:], ps2[:, :, o:],
                                         AF.Exp, scale=scale)
                    if o > op:
                        # odd diagonal tile: zero the columns its even partner
                        # covers but it does not
                        nc.vector.memset(wei8[:, 1, :, op:o], 0.0)
                    if kt >= 8 * t:
                        nc.vector.tensor_mul(wei8[:, kt & 1, :, o:o + 64],
                                             wei8[:, kt & 1, :, o:o + 64],
                                             mask_sb[:])

                sc_iter = score_items()

                def issue_next_score():
                    item = next(sc_iter, None)
                    if item is not None:
                        scores2(*item)

                issue_next_score()
                issue_next_score()
                issue_next_score()
                for t, hp in units:
                    nkt = 8 * (t + 1)
                    pavs = [psAV.tile([P, QB], F32, name=f"pav{l}", tag="pav")
                            for l in range(2)]
                    for m in range(nkt // 2):
                        issue_next_score()
                        issue_next_score()
                        op = 64 * (2 * m - 8 * t) if 2 * m >= 8 * t else 0
                        wei8 = wei8s.pop((t, hp, m))
                        for l in range(2):
                            nc.tensor.matmul(pavs[l][0:65, op:],
                                             V_all[:, 2 * m:2 * m + 2, 2 * hp + l, :],
                                             wei8[:, :, l, op:],
                                             start=(m == 0), stop=(m == nkt // 2 - 1),
                                             perf_mode=DR)
                        if m == 0:
                            flush_den()
                    pend.append((t, hp, pavs))
                flush_den()
                for i in range(DP):
                    wo_group(0, i, psAV, "pav")
            sKVQ.close()

        if KPHASES == 2:
            # debug: dump x2T (or attn8 with KDUMP=ATT) and stop
            dsrc = attn8 if os.environ.get("KDUMP") == "ATT" else x2T
            with tc.tile_pool(name="dbg", bufs=2) as dbg:
                for i in range(DP):
                    dt_ = dbg.tile([P, TQ], F32, name="dt", tag="dt")
                    nc.vector.tensor_copy(dt_[:], dsrc[:, i, :])
                    nc.sync.dma_start(out3[:, i, :], dt_[:])
            p3t.close()
            sATT.close()
            sX2.close()
            sWX.close()

        if KPHASES >= 3:
            # ================= Phase 3b/4: LN2 + FFN + residual =================
            with ExitStack() as p34:
                l2p = p34.enter_context(tc.tile_pool(name="l2p", bufs=2))
                l2r = p34.enter_context(tc.tile_pool(name="l2r", bufs=1))
                l2ps = p34.enter_context(tc.tile_pool(name="l2ps", bufs=1, space="PSUM"))
                # note: psF/psO below use bufs=3; l2ps holds 2 banks -> 8 total
                rp = p34.enter_context(tc.tile_pool(name="rp", bufs=1))
                top = p34.enter_context(tc.tile_pool(name="top", bufs=4))
                w1p = p34.enter_context(tc.tile_pool(name="w1p", bufs=2, side="right"))
                w2p = p34.enter_context(tc.tile_pool(name="w2p", bufs=2, side="right"))

                with ExitStack() as p4ps:
                    psF = p4ps.enter_context(tc.tile_pool(name="psF", bufs=3, space="PSUM"))
                    psO = p4ps.enter_context(tc.tile_pool(name="psO", bufs=3, space="PSUM"))

                    def ln2_half(qc):
                        """LN2 for query half qc -> h28[:, :, qc*CH:(qc+1)*CH].
                        DVE add-chains for the stats, two small bf16 matmuls
                        for the cross-partition reduce."""
                        sl = ts(qc, CH)
                        s1a = l2p.tile([P, CH], F32, name="s1a", tag="s1a")
                        nc.vector.tensor_add(s1a[:], x2T[:, 0, sl], x2T[:, 1, sl])
                        for i in range(2, DP - 1):
                            nc.vector.tensor_add(s1a[:], s1a[:], x2T[:, i, sl])
                        s1t = l2p.tile([P, CH], BF16, name="s1t", tag="s1t")
                        nc.vector.tensor_add(s1t[:], s1a[:], x2T[:, DP - 1, sl])
                        s2a = l2p.tile([P, CH], F32, name="s2a", tag="s2a")
                        sq0 = l2p.tile([P, CH], BF16, name="sq", tag="sq")
                        nc.scalar.activation(sq0[:], x2T[:, 0, sl], AF.Square)
                        sq1 = l2p.tile([P, CH], BF16, name="sq", tag="sq")
                        nc.scalar.activation(sq1[:], x2T[:, 1, sl], AF.Square)
                        nc.vector.tensor_add(s2a[:], sq0[:], sq1[:])
                        for i in range(2, DP - 1):
                            sqi = l2p.tile([P, CH], BF16, name="sq", tag="sq")
                            nc.scalar.activation(sqi[:], x2T[:, i, sl], AF.Square)
                            nc.vector.tensor_add(s2a[:], s2a[:], sqi[:])
                        sql = l2p.tile([P, CH], BF16, name="sq", tag="sq")
                        nc.scalar.activation(sql[:], x2T[:, DP - 1, sl], AF.Square)
                        s2t = l2p.tile([P, CH], BF16, name="s2t", tag="s2t")
                        nc.vector.tensor_add(s2t[:], s2a[:], sql[:])
                        ps_s = l2ps.tile([1, 2, CH], F32, name="ps_st", tag="ps_st")
                        nc.tensor.matmul(ps_s[:, 0, :], ones_b16[:], s1t[:],
                                         start=True, stop=True)
                        nc.tensor.matmul(ps_s[:, 1, :], ones_b16[:], s2t[:],
                                         start=True, stop=True)
                        negmu2 = l2p.tile([1, CH], F32, name="negmu2", tag="negmu2")
                        nc.vector.tensor_scalar_mul(negmu2[:], ps_s[:, 0, :], -1.0 / D)
                        muu2 = l2p.tile([1, CH], F32, name="muu2", tag="muu2")
                        nc.vector.tensor_mul(muu2[:], negmu2[:], negmu2[:])
                        sdb2 = l2p.tile([1, CH], F32, name="sdb2", tag="sdb2")
                        nc.vector.tensor_scalar(sdb2[:], muu2[:], -1.0, EPS,
                                                op0=ALU.mult, op1=ALU.add)
                        var2 = l2p.tile([1, CH], F32, name="var2", tag="var2")
                        nc.vector.scalar_tensor_tensor(var2[:], ps_s[:, 1, :],
                                                       1.0 / D, sdb2[:],
                                                       op0=ALU.mult, op1=ALU.add)
                        sd2 = l2p.tile([1, CH], F32, name="sd2", tag="sd2")
                        nc.scalar.activation(sd2[:], var2[:], AF.Sqrt)
                        rstd2 = l2p.tile([1, CH], F32, name="rstd2", tag="rstd2")
                        nc.vector.reciprocal_approx_fast(rstd2[:], sd2[:])
                        nmr2 = l2p.tile([1, CH], F32, name="nmr2", tag="nmr2")
                        nc.vector.tensor_mul(nmr2[:], negmu2[:], rstd2[:])
                        rstd2b = l2p.tile([P, CH], F32, name="rstd2b", tag="rstd2b")
                        nc.gpsimd.partition_broadcast(rstd2b[:], rstd2[:], channels=P)
                        nmr2b = l2p.tile([P, CH], F32, name="nmr2b", tag="nmr2b")
                        nc.gpsimd.partition_broadcast(nmr2b[:], nmr2[:], channels=P)
                        for i in range(DP):
                            tmp = l2p.tile([P, CH], BF16, name="h2t", tag="h2t")
                            nc.vector.tensor_mul(tmp[:], x2T[:, i, sl], rstd2b[:])
                            nc.vector.tensor_add(h28[:, i, sl], tmp[:], nmr2b[:])

                    # LN2 half 0 depends only on phase-2 output; its DVE/ACT
                    # work overlaps the wo(1) matmuls below
                    ln2_half(0)
                    for i in range(DP):
                        wo_group(1, i, psF, "psf")
                    ln2_half(1)

                    # ---- FFN: w1 in fp8 DoubleRow, w2 in bf16 (precision), with
                    # query chunks outer and a single rT buffer ----
                    for qc in range(2):
                        rT = rp.tile([P, FP, CH], BF16, name="rT", tag="rT")
                        for fs in range(8):
                            w1c = w1p.tile([P, DP, CH], FP8, name="w1c", tag="w1c")
                            nc.gpsimd.dma_start(w1c[:], w13[:, :, ts(fs, CH)])
                            for fj in range(4):
                                f = fs * 4 + fj
                                ps = psF.tile([P, CH], F32, name="psf", tag="psf")
                                for ks in range(4):
                                    nc.tensor.matmul(ps[:], w1c[:, 2 * ks:2 * ks + 2, ts(fj, P)],
                                                     h28[:, 2 * ks:2 * ks + 2, ts(qc, CH)],
                                                     start=(ks == 0), stop=(ks == 3),
                                                     perf_mode=DR)
                                nc.scalar.activation(rT[:, f, :], ps[:],
                                                     AF.Relu,
                                                     bias=bias_sb[:, 32 + f:33 + f],
                                                     scale=2.0 ** -K_1)
                        for i in range(DP):
                            w2i = w2p.tile([P, FP, P], BF16, name="w2i", tag="w2i")
                            nc.gpsimd.dma_start(w2i[:], w23[:, :, ts(i, P)])
                            ps2 = psO.tile([P, CH], F32, name="ps2", tag="ps2")
                            for f in range(FP):
                                nc.tensor.matmul(ps2[:], w2i[:, f, :], rT[:, f, :],
                                                 start=(f == 0), stop=(f == FP - 1))
                            ot = top.tile([P, CH], F32, name="ot", tag="ot")
                            nc.vector.scalar_tensor_tensor(
                                ot[:], ps2[:], bias_sb[:, 8 + i:9 + i],
                                x2T[:, i, ts(qc, CH)], op0=ALU.add, op1=ALU.add)
                            nc.sync.dma_start(out3[:, i, ts(qc, CH)], ot[:])
            p3t.close()
            sATT.close()
            sX2.close()
            sWX.close()

    nc.compile()
    return nc


def _fp8_scale(w, k):
    """Scale w by 2**k and cast to fp8e4m3 (max-normal 240), asserting range."""
    s = np.asarray(w, np.float32) * (2.0 ** k)
    assert np.abs(s).max() < 239.0, f"fp8 overflow: {np.abs(s).max()}"
    return np.ascontiguousarray(s.astype(ml_dtypes.float8_e4m3))


def prepare_inputs(x, wq, wk, wv, wo, bo, w1, b1, w2, b2,
                   g_ln1, b_ln1, g_ln2, b_ln2):
    """Host-side sharding/prep. Returns list of 8 per-core input dicts."""
    f32 = np.float32
    bf = ml_dtypes.bfloat16
    x = np.asarray(x, f32)
    g1 = np.asarray(g_ln1, f32)
    b1n = np.asarray(b_ln1, f32)
    g2 = np.asarray(g_ln2, f32)
    b2n = np.asarray(b_ln2, f32)

    wq_e = _fp8_scale(g1[:, None] * np.asarray(wq, f32), K_QKV)
    wk_e = _fp8_scale(g1[:, None] * np.asarray(wk, f32), K_QKV)
    wv_e = _fp8_scale(g1[:, None] * np.asarray(wv, f32), K_QKV)
    wo_e = _fp8_scale(np.asarray(wo, f32), K_O)
    w1_e = _fp8_scale(g2[:, None] * np.asarray(w1, f32), K_1)
    w2_e = np.ascontiguousarray(np.asarray(w2, f32).astype(bf))

    bq = b1n @ np.asarray(wq, f32)
    bk = b1n @ np.asarray(wk, f32)
    bv = b1n @ np.asarray(wv, f32)
    b1p = np.asarray(b1, f32) + b2n @ np.asarray(w1, f32)
    bo_f = np.asarray(bo, f32)
    b2_f = np.asarray(b2, f32)

    def pcol(v, n):  # [n*128] -> [128, n] partition-major
        return np.ascontiguousarray(np.asarray(v, f32).reshape(n, P).T)

    biases = np.zeros((P, 64), f32)
    biases[:, 8:16] = pcol(b2_f, 8)
    biases[:, 16:24] = pcol(bq, 8)
    biases[:, 24:32] = pcol(bk, 8)
    biases[:, 32:64] = pcol(b1p, 32)
    bvr = np.ascontiguousarray(np.broadcast_to(bv[None, :], (P, D)))

    # per-parity token permutation: queries always land on even positions
    perms = {}
    for d in (0, 1):
        perm = np.arange(TKV)
        if d == 1:
            perm = perm.reshape(-1, 2)[:, ::-1].reshape(-1)
        perms[d] = perm

    # diagonal-tile causal masks: key at tile position p (original token
    # perm[base+p]) may be attended by query column r (original token
    # perm[base+2r] = base + 2r + d) iff perm[base+p] <= base + 2r + d.
    masks = {}
    for d in (0, 1):
        pp = perms[d][:P]               # relative original positions
        r = np.arange(64)[None, :]
        m = (pp[:, None] <= (2 * r + d)).astype(bf)
        masks[d] = np.ascontiguousarray(np.broadcast_to(m[:, None, :], (P, 2, 64)))

    in_maps = []
    for c in range(8):
        b, d = divmod(c, 2)
        xo = x[b, d::2].T + bo_f[:, None]
        in_maps.append(dict(
            xn=np.ascontiguousarray(x[b][perms[d]].astype(bf)),
            xoT=np.ascontiguousarray(xo),
            wq=wq_e, wk=wk_e, wv=wv_e, wo=wo_e, w1=w1_e, w2=w2_e,
            biases=biases, bvr=bvr, mk=masks[d],
        ))
    return in_maps


_NC = None
LAST_RESULTS = None


def kernel(**inputs):
    global _NC, LAST_RESULTS
    in_maps = prepare_inputs(**inputs)
    if _NC is None:
        _NC = build_nc()
    res = run_bass_kernel_spmd(_NC, in_maps, core_ids=list(range(8)))
    LAST_RESULTS = res
    out = np.empty((4, TKV, D), np.float32)
    for c in range(8):
        b, d = divmod(c, 2)
        out[b, d::2, :] = res.results[c]["outT"].T
    return out


if __name__ == "__main__":
    z = np.load("/root/problem/ref_cache.npz")
    inputs = {k: z[k] for k in z.files if k != "out"}
    out = kernel(**inputs)
    ref = z["out"]
    err = np.abs(out - ref)
    print("abs max err:", err.max(), "scale-rel:", err.max() / np.abs(ref).max())

